# revision 9
# baseline (speedup 1.0000x reference)
"""Trainium2 Bass kernel for ConvFourierKANLayer.

Computes y = conv2d(cos(x*k), w0) + conv2d(sin(x*k), w1) + bias for
k = 1..10 (G=10 Fourier orders), 3x3 kernel, pad 1, C=64 -> O=128.

Strategy (8 NeuronCores, data-parallel over batch B=16 -> 2 per core):
  - Host pre-transposes fouriercoeffs into 90 lhsT tiles [K=128, O=128]
    where K = (g_parity, c) packs two Fourier orders per matmul, and the
    tile index t enumerates (branch, g_pair, kh, kw).
  - On-chip, x rows are expanded to cos/sin of k*x. The DVE has no fp
    mod, so the argument reduction uses the fp32 magic-number rounding
    trick (only add/sub/mult, all ISA-valid tensor_scalar ops):
        u  = x*(k/2pi) + 16        (positive)
        v  = (u + 2^23) - 2^23     (= round(u), fp32 round-to-nearest)
        w  = u - v                 (in [-0.5, 0.5])
        sin(k*x) = Sin(w * 2pi)    (ScalarE spline, valid on [-pi, pi])
    cos uses u_c = u + 0.25 (phase + pi/2) through the same pipeline.
  - Implicit GEMM: per 8-row output strip, accumulate 90 matmuls
    (branch x g_pair x 3x3 taps) of [K=128]x[O=128] @ [K=128, N=512]
    into one PSUM bank, with float32r (full-rate fp22) arithmetic.
"""

import numpy as np

import concourse.bass as bass
import concourse.mybir as mybir
import concourse.tile as tile
from concourse import bacc
from concourse.bass_utils import run_bass_kernel_spmd

N_CORES = 8
B, C, H, W = 16, 64, 64, 64
O = 128
G = 10
BS = B // N_CORES  # batches per core
HT = 16  # output rows per chunk
NB = HT // 8  # psum banks (8-row strips) per chunk
NT = 2 * 5 * 9  # weight tiles: branch x g_pair x 3 x 3

PI = float(np.pi)
TWO_PI = float(2 * np.pi)
MAGIC = 8388608.0  # 2^23: fp32 round-to-nearest-integer magic constant

F32 = mybir.dt.float32
F32R = mybir.dt.float32r

_CACHE = {}


def _build_module(reps=1):
    nc = bacc.Bacc("TRN2", target_bir_lowering=False)
    x_d = nc.dram_tensor("x", [BS, C, H, W], F32, kind="ExternalInput")
    w_d = nc.dram_tensor("w", [128, NT, 128], F32R, kind="ExternalInput")
    kv_d = nc.dram_tensor("kvec", [128, 5], F32, kind="ExternalInput")
    bias_d = nc.dram_tensor("biasv", [128, 1], F32, kind="ExternalInput")
    y_d = nc.dram_tensor("y", [BS, O, H, W], F32, kind="ExternalOutput")

    mult = mybir.AluOpType.mult
    add = mybir.AluOpType.add
    sin_f = mybir.ActivationFunctionType.Sin

    with tile.TileContext(nc) as tc:
        with (
            tc.tile_pool(name="const", bufs=1) as cpool,
            tc.tile_pool(name="wpool", bufs=1) as wpool,
            tc.tile_pool(name="gen", bufs=2) as gen,
            tc.tile_pool(name="cspool", bufs=3) as cspool,
            tc.tile_pool(name="outp", bufs=3) as outp,
            tc.tile_pool(name="psum", bufs=4, space="PSUM") as psum,
        ):
            wt = wpool.tile([128, NT, 128], F32R)
            nc.sync.dma_start(wt[:], w_d[:])
            kvt = cpool.tile([128, 5], F32)
            nc.sync.dma_start(kvt[:], kv_d[:])
            bt = cpool.tile([128, 1], F32)
            nc.sync.dma_start(bt[:], bias_d[:])

            for rep in range(reps):
              for b in range(BS):
                for h0 in range(0, H, HT):
                    gr0, gr1 = max(0, h0 - 1), min(H, h0 + HT + 1)
                    l0 = gr0 - (h0 - 1)  # local row index of first real row
                    nrows = gr1 - gr0
                    rs = slice(l0, l0 + nrows)

                    xd = gen.tile([128, HT + 2, W], F32, tag="xdup")
                    nc.sync.dma_start(xd[0:64, rs, :], x_d[b, :, gr0:gr1, :])
                    nc.sync.dma_start(xd[64:128, rs, :], x_d[b, :, gr0:gr1, :])

                    pss = [
                        psum.tile([128, 8, 64], F32, tag=f"ps{bk}",
                                  name=f"ps{bk}_{rep}_{b}_{h0}")
                        for bk in range(NB)
                    ]

                    for j in range(5):
                        # u = x*(k/2pi) + 16 ; v = round(u) ; w = u - v
                        us = gen.tile([128, HT + 2, W], F32, tag="us")
                        nc.vector.tensor_scalar(
                            us[:, rs, :], xd[:, rs, :],
                            kvt[:, j : j + 1], 16.0, mult, add,
                        )
                        uc = gen.tile([128, HT + 2, W], F32, tag="uc")
                        nc.vector.tensor_scalar_add(uc[:, rs, :], us[:, rs, :], 0.25)

                        st = cspool.tile([128, HT + 2, W + 2], F32R, tag="ss")
                        ct = cspool.tile([128, HT + 2, W + 2], F32R, tag="cs")
                        for u_t, z in ((us, st), (uc, ct)):
                            v_t = gen.tile([128, HT + 2, W], F32, tag="vt")
                            nc.vector.tensor_scalar_add(
                                v_t[:, rs, :], u_t[:, rs, :], MAGIC
                            )
                            nc.vector.tensor_scalar_sub(
                                v_t[:, rs, :], v_t[:, rs, :], MAGIC
                            )
                            w_t = gen.tile([128, HT + 2, W], F32, tag="wt")
                            nc.vector.tensor_sub(
                                w_t[:, rs, :], u_t[:, rs, :], v_t[:, rs, :]
                            )
                            # zero borders (uint32 bitcast: memset can't
                            # encode fp32r), then fill interior with Sin
                            u32 = mybir.dt.uint32
                            nc.vector.memset(z[:, :, 0:1].bitcast(u32), 0)
                            nc.vector.memset(z[:, :, W + 1 : W + 2].bitcast(u32), 0)
                            if l0 == 1:
                                nc.vector.memset(z[:, 0:1, :].bitcast(u32), 0)
                            if gr1 == H:
                                nc.vector.memset(
                                    z[:, HT + 1 : HT + 2, :].bitcast(u32), 0
                                )
                            nc.scalar.activation(
                                z[:, rs, 1 : W + 1], w_t[:, rs, :], sin_f,
                                scale=TWO_PI,
                            )

                        for br in range(2):
                            src = ct if br == 0 else st
                            for dh in range(3):
                                for dw in range(3):
                                    t_idx = ((br * 5 + j) * 3 + dh) * 3 + dw
                                    for bk in range(NB):
                                        nc.tensor.matmul(
                                            pss[bk][:],
                                            wt[:, t_idx, :],
                                            src[
                                                :,
                                                8 * bk + dh : 8 * bk + dh + 8,
                                                dw : dw + 64,
                                            ],
                                            start=(j == 0 and br == 0
                                                   and dh == 0 and dw == 0),
                                            stop=(j == 4 and br == 1
                                                  and dh == 2 and dw == 2),
                                        )

                    for bk in range(NB):
                        ob = outp.tile([128, 8, 64], F32, tag="ob")
                        nc.vector.tensor_scalar_add(ob[:], pss[bk][:], bt[:, 0:1])
                        nc.sync.dma_start(
                            y_d[b, :, h0 + 8 * bk : h0 + 8 * bk + 8, :], ob[:]
                        )
    nc.finalize()
    return nc


def _get_module(reps=1):
    key = ("nc", reps)
    if key not in _CACHE:
        _CACHE[key] = _build_module(reps)
    return _CACHE[key]


def _host_weights(fc):
    # fc: (2, O, C, kH, kW, G) -> w[p=(gp*64+c), t=(br,j,kh,kw), o]
    W6 = np.transpose(fc, (0, 5, 3, 4, 2, 1))  # (br, g, kh, kw, c, o)
    W6 = W6.reshape(2, 5, 2, 3, 3, 64, 128)  # (br, j, gp, kh, kw, c, o)
    Wt = np.transpose(W6, (0, 1, 3, 4, 2, 5, 6))  # (br, j, kh, kw, gp, c, o)
    Wt = Wt.reshape(NT, 128, 128)
    return np.ascontiguousarray(np.transpose(Wt, (1, 0, 2)), dtype=np.float32)


def _host_kvec():
    kvec = np.zeros((128, 5), np.float32)
    for j in range(5):
        kvec[0:64, j] = (2 * j + 1) / TWO_PI
        kvec[64:128, j] = (2 * j + 2) / TWO_PI
    return kvec


def kernel(x, fouriercoeffs, bias):
    x = np.ascontiguousarray(np.asarray(x, dtype=np.float32))
    fc = np.asarray(fouriercoeffs, dtype=np.float32)
    w_host = _host_weights(fc)
    kvec = _host_kvec()
    biasv = np.ascontiguousarray(
        np.asarray(bias, dtype=np.float32).reshape(128, 1)
    )

    nc = _get_module()
    in_maps = [
        {"x": x[i * BS : (i + 1) * BS], "w": w_host, "kvec": kvec, "biasv": biasv}
        for i in range(N_CORES)
    ]
    res = run_bass_kernel_spmd(nc, in_maps, list(range(N_CORES))).results
    return np.concatenate([res[i]["y"] for i in range(N_CORES)], axis=0)
